# revision 3
# baseline (speedup 1.0000x reference)
"""Trainium2 Bass kernel for nn_LongRangeFeaturizer (Ewald sum).

v4 -> v5 (throughput-oriented consolidation; HW-calibrated op costs):
- ONE input DMA (all inputs packed into a single f16 tensor, i16 bitcast)
- joint-2-system trig ([128,1024] ops amortize ~310ns/op scalar overhead)
- merged charges / S(k) / pot psums across systems (partition or column
  offsets); single combine stt + single output DMA
- SR chain unsplit (W-wide ops), 4 per-jt scans feeding 4 paired scatters
"""

import sys

sys.path.insert(0, "/opt/trn_rl_repo")

import numpy as np

import concourse.bass as bass
import concourse.mybir as mybir
import concourse.tile as tile
from concourse import bacc, bass_utils

dt = mybir.dt
F32, F16, I16 = dt.float32, dt.float16, dt.int16
AF = mybir.ActivationFunctionType
AOP = mybir.AluOpType

PI = float(np.pi)
MAGIC = float(1.5 * 2**23)
S2PI = float(np.nextafter(np.float32(2 * np.pi), np.float32(0)))
A2 = float(2.0 / np.sqrt(np.pi))   # erf(x) ~ tanh(A2 x + B2 x^3)
B2 = 0.10185562
DIAG_D = 1e-4
DUMMY_ACT = AF.Silu  # simtest overrides (interp lacks Silu)

S, N, D, E = 16, 512, 64, 16384
LCELL = 8.0
SMEAR = 1.0
EXCL = 5.0
PREF = 1.0
NSQ_MAX = 16
NCORES = 8
SC = S // NCORES

_CACHE = {}


def _half_kgrid():
    r = np.arange(-4, 5)
    n = np.stack(np.meshgrid(r, r, r, indexing="ij"), -1).reshape(-1, 3)
    n = n[np.any(n != 0, axis=1)]
    n = n[(n * n).sum(1) <= NSQ_MAX]
    pos = (n[:, 0] > 0) | ((n[:, 0] == 0) & (n[:, 1] > 0)) | (
        (n[:, 0] == 0) & (n[:, 1] == 0) & (n[:, 2] > 0))
    return n[pos].astype(np.int64)


def _sr_arrange_pairs(nidx, ndist):
    """Paired scatter layout with anti-matched sys1 atom permutation.

    Slot j holds sys0 atom j and sys1 atom perm1[j], chosen so heavy sys0
    rows pair with light sys1 rows (minimizes the padded row width R2).
    Returns per-core (Dd, Ii, Mm), per-core perm1, R2."""
    pre = []
    R2 = 0
    for core in range(NCORES):
        s0 = core * SC
        cnt0 = np.bincount(nidx[s0, :, 1].astype(np.int64),
                           minlength=N) + 1
        cnt1 = np.bincount(nidx[s0 + 1, :, 1].astype(np.int64),
                           minlength=N) + 1
        slot_order = np.argsort(-cnt0, kind="stable")
        atom_order = np.argsort(cnt1, kind="stable")
        perm1 = np.empty(N, np.int64)
        perm1[slot_order] = atom_order
        inv1 = np.empty(N, np.int64)
        inv1[perm1] = np.arange(N)
        R2 = max(R2, int((cnt0 + cnt1[perm1]).max()))
        pre.append((s0, perm1, inv1))
    R2 = R2 + (R2 % 2)

    out, perms = [], []
    for s0, perm1, inv1 in pre:
        perms.append(perm1)
        Dd = np.full((128, 4 * R2), 5.0, np.float16)
        Ii = np.full((128, 4 * R2), -1, np.int16)
        Mm = np.ones((128, 4 * R2), np.float16)
        sys_rows = []
        for si in range(SC):
            s = s0 + si
            j_all = np.concatenate([nidx[s, :, 1].astype(np.int64),
                                    np.arange(N, dtype=np.int64)])
            i_all = np.concatenate([nidx[s, :, 0].astype(np.int64),
                                    np.arange(N, dtype=np.int64)])
            if si == 1:
                j_all = inv1[j_all]
                i_all = inv1[i_all]
            d_all = np.concatenate([ndist[s].astype(np.float64),
                                    np.full(N, DIAG_D)])
            cidx = j_all * N + i_all
            order = np.argsort(cidx, kind="stable")
            cs, ds = cidx[order], d_all[order]
            js = cs // N
            newseg = np.ones(len(cs), bool)
            newseg[1:] = cs[1:] != cs[:-1]
            lastseg = np.ones(len(cs), bool)
            lastseg[:-1] = newseg[1:]
            cnt = np.bincount(js, minlength=N)
            start = np.concatenate([[0], np.cumsum(cnt)[:-1]])
            rows = []
            for j in range(N):
                sl = slice(start[j], start[j] + cnt[j])
                rows.append((ds[sl], (cs % N)[sl], newseg[sl], lastseg[sl]))
            sys_rows.append(rows)
        for jt in range(4):
            for p in range(128):
                j = jt * 128 + p
                c0 = jt * R2
                off = 0
                for si in range(SC):
                    ds, is_, newseg, lastseg = sys_rows[si][j]
                    nsl = len(ds)
                    Dd[p, c0 + off:c0 + off + nsl] = ds.astype(np.float16)
                    Ii[p, c0 + off:c0 + off + nsl] = np.where(
                        lastseg, is_ + si * N, -1).astype(np.int16)
                    Mm[p, c0 + off:c0 + off + nsl] = (~newseg).astype(
                        np.float16)
                    off += nsl
        out.append((Dd, Ii, Mm))
    return out, perms, R2


def _offsets(W):
    o = {}
    o["SRD"] = 0
    o["MSK"] = W
    o["SRI"] = 2 * W
    o["ID"] = 3 * W
    o["G"] = 3 * W + 128
    o["NEG"] = 3 * W + 130
    o["NT"] = 3 * W + 258
    o["WT"] = 3 * W + 386
    o["PT"] = 3 * W + 450
    o["FEAT"] = 3 * W + 1474
    o["PW"] = 3 * W + 2498
    return o


def _build_nc(K, R2, reps=1, unroll=1):
    nc = bacc.Bacc("TRN2", target_bir_lowering=False, debug=False,
                   num_devices=NCORES)

    for val in (PI / 2,):
        t = nc.alloc_sbuf_tensor(f"constap-{val}", [128, 1], F32)
        nc.gpsimd.memset(t.ap(), val)
        nc.const_aps.aps[(F32, val)] = t.ap()
    nc.all_engine_barrier()
    t_dum = nc.alloc_sbuf_tensor("actdummy", [128, 1], F32)
    nc.scalar.activation(t_dum.ap(), nc.const_aps.aps[(F32, PI / 2)],
                         DUMMY_ACT)
    nc.all_engine_barrier()

    W = 4 * R2
    O = _offsets(W)
    pk = nc.dram_tensor("pk", [128, O["PW"]], F16, kind="ExternalInput").ap()
    out = nc.dram_tensor("out", [128, N], F32, kind="ExternalOutput").ap()

    bgov = PREF * float(PI * SMEAR**2 / (LCELL**3))

    from contextlib import nullcontext
    with tile.TileContext(nc) as tc:
        with (
            tc.tile_pool(name="const", bufs=2) as cp,
            tc.tile_pool(name="work", bufs=2) as wp,
            tc.tile_pool(name="trig", bufs=1) as tp,
            tc.tile_pool(name="psph", bufs=1, space="PSUM") as pph,
            tc.tile_pool(name="psmm", bufs=1, space="PSUM") as pmm,
            tc.tile_pool(name="psnk", bufs=2, space="PSUM") as pnk,
            tc.tile_pool(name="pss2", bufs=1, space="PSUM") as ps2,
            tc.tile_pool(name="pspot", bufs=2, space="PSUM") as ppot,
            tc.For_i(0, reps, 1) if reps > 1 else nullcontext(),
        ):
          for _u in range(unroll):
            t_P = cp.tile([128, O["PW"]], F16, tag="pk")
            nc.sync.dma_start(out=t_P[:], in_=pk[:])

            t_srd = t_P[:, O["SRD"]:O["SRD"] + W]
            t_mask = t_P[:, O["MSK"]:O["MSK"] + W]
            t_sri = t_P[:, O["SRI"]:O["SRI"] + W].bitcast(I16)
            t_id = t_P[:, O["ID"]:O["ID"] + 128]
            t_Gc = t_P[:, O["G"]:O["G"] + 2].bitcast(F32)
            t_negI = t_P[:, O["NEG"]:O["NEG"] + 128]
            t_nt6 = t_P[0:6, O["NT"]:O["NT"] + 128]
            t_WT = t_P[0:65, O["WT"]:O["WT"] + 64]
            t_pT6 = t_P[0:6, O["PT"]:O["PT"] + 1024]
            t_feat = t_P[0:65, O["FEAT"]:O["FEAT"] + 1024]

            # ---- SR chain (W-wide), 4 scans, 4 paired scatters ----
            t_d2 = wp.tile([128, W], F32, tag="d2")
            nc.scalar.activation(t_d2[:], t_srd, AF.Square,
                                 scale=float(np.sqrt(B2 / 2.0)))
            t_r = wp.tile([128, W], F32, tag="rec")
            nc.vector.reciprocal(t_r[:], t_srd)
            t_arg = wp.tile([128, W], F32, tag="arg")
            nc.vector.scalar_tensor_tensor(
                out=t_arg[:], in0=t_d2[:], scalar=float(A2 / np.sqrt(2.0)),
                in1=t_srd, op0=AOP.add, op1=AOP.mult)
            t_e = wp.tile([128, W], F32, tag="erf")
            nc.scalar.activation(t_e[:], t_arg[:], AF.Tanh)
            t_v = wp.tile([128, W], F32, tag="fv")
            nc.scalar.activation(t_v[:], t_srd, AF.Sin,
                                 scale=float(-PI / 10.0), bias=PI / 2)
            t_f2 = wp.tile([128, W], F32, tag="f2")
            nc.scalar.activation(t_f2[:], t_v[:], AF.Square)
            t_er = wp.tile([128, W], F32, tag="er")
            nc.vector.tensor_tensor(out=t_er[:], in0=t_e[:], in1=t_r[:],
                                    op=AOP.mult)
            t_sr = wp.tile([128, W], F32, tag="sr")
            nc.vector.scalar_tensor_tensor(
                out=t_sr[:], in0=t_f2[:], scalar=-1.0, in1=t_er[:],
                op0=AOP.mult, op1=AOP.mult)
            t_srm = []
            for jt in range(4):
                js = slice(jt * R2, (jt + 1) * R2)
                t_s = wp.tile([128, R2], F16, tag=f"srm{jt}", name=f"srm{jt}")
                nc.vector.tensor_tensor_scan(
                    out=t_s[:], data0=t_mask[:, js], data1=t_sr[:, js],
                    initial=0.0, op0=AOP.mult, op1=AOP.add)
                t_srm.append(t_s)
            t_M = []
            for jt in range(4):
                t_m = tp.tile([128, 2 * N], F16, tag=f"mt{jt}", name=f"mt{jt}")
                nc.gpsimd.local_scatter(
                    out_ap=t_m[:], data_ap=t_srm[jt][:],
                    idxs_ap=t_sri[:, jt * R2:(jt + 1) * R2],
                    channels=128, num_elems=2 * N, num_idxs=R2)
                t_M.append(t_m)

            # ---- joint trig: phase -> round -> sin / abs / cos ----
            ps_u = pph.tile([128, 2 * N], F32, tag="phase")
            for c in range(2):
                cs_ = slice(c * N, c * N + N)
                nc.tensor.matmul(out=ps_u[:, cs_], lhsT=t_nt6,
                                 rhs=t_pT6[:, cs_], start=True, stop=True)
            t_i16 = wp.tile([128, 2 * N], F16, tag="i16")
            nc.vector.tensor_scalar(out=t_i16[:], in0=ps_u[:],
                                    scalar1=MAGIC, scalar2=MAGIC,
                                    op0=AOP.add, op1=AOP.subtract)
            for c in range(2):
                cs_ = slice(c * N, c * N + N)
                nc.tensor.matmul(out=ps_u[:, cs_], lhsT=t_negI,
                                 rhs=t_i16[:, cs_], start=False, stop=True,
                                 skip_group_check=True)
            t_skn = tp.tile([128, 2 * N], F16, tag="skn")
            nc.scalar.activation(t_skn[:], ps_u[:], AF.Sin, scale=S2PI)
            t_ra = wp.tile([128, 2 * N], F32, tag="ra")
            nc.scalar.activation(t_ra[:], ps_u[:], AF.Abs)
            t_ckn = tp.tile([128, 2 * N], F16, tag="ckn")
            nc.scalar.activation(t_ckn[:], t_ra[:], AF.Sin, scale=-S2PI,
                                 bias=PI / 2)

            # ---- NK via PE transposes (8 blocks per comp), 1 copy each ----
            ps_sT = pnk.tile([128, 2 * N], F16, tag="nk")
            for b in range(8):
                bs = slice(b * 128, b * 128 + 128)
                nc.tensor.transpose(out=ps_sT[:, bs], in_=t_skn[:, bs],
                                    identity=t_id)
            t_snk = tp.tile([128, 2 * N], F16, tag="snk")
            nc.vector.tensor_copy(out=t_snk[:], in_=ps_sT[:])
            ps_cT = pnk.tile([128, 2 * N], F16, tag="nk")
            for b in range(8):
                bs = slice(b * 128, b * 128 + 128)
                nc.tensor.transpose(out=ps_cT[:, bs], in_=t_ckn[:, bs],
                                    identity=t_id)
            t_cnk = tp.tile([128, 2 * N], F16, tag="cnk")
            nc.vector.tensor_copy(out=t_cnk[:], in_=ps_cT[:])

            # ---- charges: merged [128,512] psums; q16 pair [128,512] ----
            ps_qTp = pmm.tile([128, N], F32, tag="mm")
            for sys in range(SC):
                csl = slice(sys * N, sys * N + N)
                nc.tensor.matmul(out=ps_qTp[sys * 64:sys * 64 + 64, :],
                                 lhsT=t_WT, rhs=t_feat[:, csl],
                                 start=True, stop=True)
            t_qTp = tp.tile([128, N], F32, tag="qTp")
            t_sumP = tp.tile([128, 1], F32, tag="sumP")
            nc.vector.tensor_scalar(out=t_qTp[:], in0=ps_qTp[:],
                                    scalar1=0.0, scalar2=0.0,
                                    op0=AOP.add, op1=AOP.add,
                                    accum_out=t_sumP[:])
            nc.vector.tensor_scalar(out=t_sumP[:], in0=t_sumP[:],
                                    scalar1=bgov, scalar2=None, op0=AOP.mult)
            ps_q4 = pmm.tile([128, 8 * 64], F32, tag="mm")
            for sys in range(SC):
                for nt_i in range(4):
                    fsl = slice(sys * N + nt_i * 128,
                                sys * N + nt_i * 128 + 128)
                    b = sys * 4 + nt_i
                    nc.tensor.matmul(out=ps_q4[:, b * 64:b * 64 + 64],
                                     lhsT=t_feat[:, fsl], rhs=t_WT,
                                     start=True, stop=True)
            t_q16 = tp.tile([128, 8 * 64], F16, tag="q16")
            nc.scalar.activation(t_q16[:], ps_q4[:], AF.Copy)

            # ---- stage1 both systems into [128, 256]; G per-partition ----
            ps_S2 = ps2.tile([128, 256], F32, tag="s2")
            for sys in range(SC):
                for comp, t_nksrc in ((0, t_cnk), (1, t_snk)):
                    col = sys * 128 + comp * 64
                    for nt_i in range(4):
                        st, sp = nt_i == 0, nt_i == 3
                        bs = slice(sys * N + nt_i * 128,
                                   sys * N + nt_i * 128 + 128)
                        qsl = slice((sys * 4 + nt_i) * 64,
                                    (sys * 4 + nt_i) * 64 + 64)
                        nc.tensor.matmul(out=ps_S2[:, col:col + 64],
                                         lhsT=t_nksrc[:, bs],
                                         rhs=t_q16[:, qsl],
                                         start=st, stop=sp)
            t_GS = tp.tile([128, 256], F16, tag="gs")
            nc.vector.tensor_scalar(out=t_GS[:], in0=ps_S2[:],
                                    scalar1=t_Gc, scalar2=None, op0=AOP.mult)

            # ---- stage2 + M@q (merged [128,512] pot), combine, 1 DMA ----
            ps_pot = ppot.tile([128, N], F32, tag="pot")
            for sys in range(SC):
                osl = slice(sys * 64, sys * 64 + 64)
                csl = slice(sys * N, sys * N + N)
                nc.tensor.matmul(out=ps_pot[osl, :],
                                 lhsT=t_GS[:, sys * 128:sys * 128 + 64],
                                 rhs=t_ckn[:, csl], start=True, stop=False)
                nc.tensor.matmul(out=ps_pot[osl, :],
                                 lhsT=t_GS[:, sys * 128 + 64:sys * 128 + 128],
                                 rhs=t_skn[:, csl], start=False, stop=False)
                for jt in range(4):
                    qsl = slice((sys * 4 + jt) * 64, (sys * 4 + jt) * 64 + 64)
                    nc.tensor.matmul(out=ps_pot[osl, :],
                                     lhsT=t_q16[:, qsl],
                                     rhs=t_M[jt][:, csl], start=False,
                                     stop=(jt == 3))
            t_out = tp.tile([128, N], F32, tag="out")
            nc.vector.scalar_tensor_tensor(
                out=t_out[:], in0=ps_pot[:], scalar=t_sumP[:, 0:1],
                in1=t_qTp[:], op0=AOP.subtract, op1=AOP.mult)
            nc.sync.dma_start(out=out[:], in_=t_out[:])

    nc.compile()
    return nc


def _host_inputs(features, positions, cells, neighbor_indices,
                 neighbor_distances, W, b):
    features = np.asarray(features, np.float32)
    positions = np.asarray(positions, np.float32)
    cells = np.asarray(cells, np.float32)
    nidx = np.asarray(neighbor_indices)
    ndist = np.asarray(neighbor_distances, np.float32).reshape(S, E)
    Wm = np.asarray(W, np.float32)
    bv = np.asarray(b, np.float32)

    assert np.allclose(cells, LCELL * np.eye(3, dtype=np.float32)[None]), \
        "kernel specialized to cubic L=8 cells"

    nh = _half_kgrid()
    K = len(nh)
    assert K <= 128
    ksq = (2.0 * PI / LCELL) ** 2 * (nh * nh).sum(1).astype(np.float64)
    vol = LCELL ** 3
    G = 2.0 * PREF * (4.0 * PI / ksq) * np.exp(-0.5 * SMEAR**2 * ksq) / vol
    Gcol = np.zeros(128, np.float64)
    Gcol[:K] = G

    pairs, perms, R2 = _sr_arrange_pairs(nidx, ndist)
    Wd = 4 * R2
    O = _offsets(Wd)

    nt6 = np.zeros((6, 128), np.float16)
    nt6[0:3, :K] = nh.T.astype(np.float16)
    nt6[3:6, :K] = nh.T.astype(np.float16)
    WT_aug = np.concatenate([Wm.T, bv[None, :]], 0).astype(np.float16)

    in_maps = []
    for core in range(NCORES):
        s0 = core * SC
        pkv = np.zeros((128, O["PW"]), np.float16)
        Dd, Ii, Mm = pairs[core]
        pkv[:, O["SRD"]:O["SRD"] + Wd] = Dd
        pkv[:, O["MSK"]:O["MSK"] + Wd] = Mm
        pkv[:, O["SRI"]:O["SRI"] + Wd] = Ii.view(np.float16)
        pkv[:, O["ID"]:O["ID"] + 128] = np.eye(128, dtype=np.float16)
        pkv[:, O["G"]:O["G"] + 2] = Gcol.astype(np.float32).reshape(
            -1, 1).view(np.float16)
        pkv[:, O["NEG"]:O["NEG"] + 128] = -np.eye(128, dtype=np.float16)
        pkv[0:6, O["NT"]:O["NT"] + 128] = nt6
        pkv[0:65, O["WT"]:O["WT"] + 64] = WT_aug
        for si in range(SC):
            s = s0 + si
            pos_s = positions[s]
            feat_s = features[s * N:(s + 1) * N]
            if si == 1:
                pos_s = pos_s[perms[core]]
                feat_s = feat_s[perms[core]]
            pf = (pos_s.T.astype(np.float64)) / LCELL
            ph = pf.astype(np.float16)
            pl = (pf - ph.astype(np.float64)).astype(np.float16)
            pkv[0:3, O["PT"] + si * N:O["PT"] + si * N + N] = ph
            pkv[3:6, O["PT"] + si * N:O["PT"] + si * N + N] = pl
            f = feat_s.T.astype(np.float16)
            pkv[0:64, O["FEAT"] + si * N:O["FEAT"] + si * N + N] = f
            pkv[64, O["FEAT"] + si * N:O["FEAT"] + si * N + N] = 1.0
        in_maps.append({"pk": pkv})
    return in_maps, K, R2, perms


def kernel(features, positions, cells, neighbor_indices, neighbor_distances,
           W, b, _trace=False):
    in_maps, K, R2, perms = _host_inputs(features, positions, cells,
                                         neighbor_indices,
                                         neighbor_distances, W, b)
    key = (K, R2)
    if key not in _CACHE:
        _CACHE[key] = _build_nc(K, R2)
    nc = _CACHE[key]
    res = bass_utils.run_bass_kernel_spmd(nc, in_maps,
                                          core_ids=list(range(NCORES)),
                                          trace=_trace)
    blocks = []
    for i in range(NCORES):
        o = res.results[i]["out"]
        for sys in range(SC):
            blk = o[sys * 64:(sys + 1) * 64, :].T
            if sys == 1:
                unp = np.empty_like(blk)
                unp[perms[i]] = blk
                blk = unp
            blocks.append(blk)
    out = np.concatenate(blocks, 0)
    if _trace:
        kernel.last_result = res
    return np.ascontiguousarray(out, dtype=np.float32)


def measure_hw_ns(features, positions, cells, neighbor_indices,
                  neighbor_distances, W, b, reps=300):
    import time
    import jax
    from jax.sharding import Mesh, PartitionSpec, NamedSharding
    from jax.experimental.shard_map import shard_map
    from concourse import bass2jax
    from concourse.bass2jax import _bass_exec_p, partition_id_tensor

    bass2jax.install_neuronx_cc_hook()
    in_maps, K, R2, _perms = _host_inputs(features, positions, cells,
                                          neighbor_indices,
                                          neighbor_distances, W, b)

    def build_fn(nc, mesh, sh):
        partition_name = (nc.partition_id_tensor.name
                          if nc.partition_id_tensor else None)
        in_names, out_names, out_avals, zero_outs = [], [], [], []
        for alloc in nc.m.functions[0].allocations:
            if not isinstance(alloc, mybir.MemoryLocationSet):
                continue
            name = alloc.memorylocations[0].name
            if alloc.kind == "ExternalInput":
                if name != partition_name:
                    in_names.append(name)
            elif alloc.kind == "ExternalOutput":
                shape = tuple(alloc.tensor_shape)
                dtype = mybir.dt.np(alloc.dtype)
                out_names.append(name)
                out_avals.append(jax.core.ShapedArray(shape, dtype))
                zero_outs.append(np.zeros(shape, dtype))
        n_params = len(in_names)
        all_names = in_names + out_names
        if partition_name is not None:
            all_names = all_names + [partition_name]

        def _body(*args):
            operands = list(args)
            if partition_name is not None:
                operands.append(partition_id_tensor())
            return tuple(_bass_exec_p.bind(
                *operands, out_avals=tuple(out_avals), in_names=tuple(all_names),
                out_names=tuple(out_names), lowering_input_output_aliases=(),
                sim_require_finite=True, sim_require_nnan=True, nc=nc))

        specs_in = (PartitionSpec("core"),) * (n_params + len(out_names))
        specs_out = (PartitionSpec("core"),) * len(out_names)
        fn = jax.jit(shard_map(_body, mesh=mesh, in_specs=specs_in,
                               out_specs=specs_out, check_rep=False),
                     keep_unused=True)
        cat = [np.concatenate([np.asarray(in_maps[c][in_names[i]])
                               for c in range(NCORES)], 0)
               for i in range(n_params)]
        cat += [np.zeros((NCORES * z.shape[0], *z.shape[1:]), z.dtype)
                for z in zero_outs]
        dev = [jax.device_put(a, sh) for a in cat]
        return fn, dev

    devices = jax.devices()[:NCORES]
    mesh = Mesh(np.asarray(devices), ("core",))
    sh = NamedSharding(mesh, PartitionSpec("core"))

    def time_min(fn, dev, n=8):
        o = fn(*dev); jax.block_until_ready(o)
        best = float("inf")
        for _ in range(n):
            t0 = time.perf_counter()
            o = fn(*dev); jax.block_until_ready(o)
            best = min(best, (time.perf_counter() - t0) * 1e9)
        return best

    key1 = (K, R2)
    if key1 not in _CACHE:
        _CACHE[key1] = _build_nc(K, R2)
    fn1, dev1 = build_fn(_CACHE[key1], mesh, sh)
    t1 = time_min(fn1, dev1)
    U = 8
    nloop = max(2, reps // U)
    keyr = (K, R2, nloop, U)
    if keyr not in _CACHE:
        _CACHE[keyr] = _build_nc(K, R2, reps=nloop, unroll=U)
    fnr, devr = build_fn(_CACHE[keyr], mesh, sh)
    tr = time_min(fnr, devr)
    return (tr - t1) / (nloop * U - 1)
